# revision 11
# baseline (speedup 1.0000x reference)
"""Trainium2 Bass kernel for a dense transformer block (pre-LN, causal MHA + GELU MLP).

Reference computation (per batch element b, all fp32):
    h   = LN(x; ln1_g, ln1_b)
    q,k,v = h @ wq, h @ wk, h @ wv       (16 heads of dim 64)
    att = softmax(causal(q k^T / 8)) v   -> [T, E]
    out = x + att @ w_proj + b_proj
    mlp = gelu(LN(out; ln2_g, ln2_b) @ w1 + b1) @ w2 + b2
    ret = out + mlp

Sharding: data-parallel over batch. B == 8 == n_cores, one batch element per
NeuronCore, no collectives. Each core runs the identical program on x[b].

Kernel layout strategy (per core):
  - LN1/LN2 computed in token layout [t, E] (free-dim reductions via bn_stats),
    then tiles are PE-transposed to e-partition layout hT/h2T [E, T], which is
    what every matmul needs (contraction dim on partitions).
  - All big matmuls use float32r (TF32-like; 1 cyc/row at N>=256, ~1e-4 relerr).
  - Attention is computed transposed: scoresT[t_k, t_q] = k_h q_h^T so that the
    softmax denominator (sum over keys) can be produced by appending a ones
    column to v_h: attT_psum[65, t_q] = [v_h | 1]^T @ exp(scoresT).  Row 64 is
    the denominator; its reciprocal is partition-broadcast and multiplied in.
  - attnT head-pair tiles (partition = 2x64 head dims) feed the proj matmul as
    the stationary operand directly; proj output lands in token layout and is
    added to x in place (residual).  The MLP's first matmul produces uT [f, t]
    (transposed), so gelu's bias b1 is a per-partition ACT bias, and the second
    matmul consumes gelu(uT) as stationary, producing token-layout output that
    is added to the residual.
"""

import numpy as np

B, T, E = 8, 1024, 1024
NH, HD, FF = 16, 64, 4096
NPAIR = NH // 2          # 8 head pairs (2 heads per 128-partition tile)
EPS = 1e-5
NCORES = 8
TCH = T // 128           # 8 token chunks
ECH = E // 128           # 8 embedding chunks
FCH = FF // 128          # 32 mlp hidden chunks
TQW = 512                # moving-dim width for t
NTQ = T // TQW           # 2

_STAGES = {"ln": 1, "vqk": 2, "attn": 3, "proj": 4, "full": 5}


def _build_program(flags, stage="full"):
    """Build + compile the SPMD Bass program.

    flags: dict of bools controlling optional bias/gain application.
    stage: truncate the program after this phase and emit debug outputs.
    """
    import concourse.bass as bass
    import concourse.tile as tile
    from concourse import bacc, mybir
    from concourse.masks import make_identity, make_upper_triangular

    sn = _STAGES[stage]
    f32 = mybir.dt.float32
    f32r = mybir.dt.float32r
    AF = mybir.ActivationFunctionType

    nc = bacc.Bacc("TRN2", target_bir_lowering=False, debug=False,
                   num_devices=NCORES)

    x_d = nc.dram_tensor("x", [T, E], f32, kind="ExternalInput").ap()
    wq_d = nc.dram_tensor("wq", [E, E], f32r, kind="ExternalInput").ap()
    wk_d = nc.dram_tensor("wk", [E, E], f32r, kind="ExternalInput").ap()
    wv_d = nc.dram_tensor("wv", [E, E], f32r, kind="ExternalInput").ap()
    wp_d = nc.dram_tensor("w_proj", [E, E], f32r, kind="ExternalInput").ap()
    w1_d = nc.dram_tensor("w1", [E, FF], f32r, kind="ExternalInput").ap()
    w2_d = nc.dram_tensor("w2", [FF, E], f32r, kind="ExternalInput").ap()
    b1_d = nc.dram_tensor("b1", [FF], f32, kind="ExternalInput").ap()
    ln1g_d = ln1b_d = ln2g_d = ln2b_d = bp_d = b2_d = None
    if flags["ln1_gb"]:
        ln1g_d = nc.dram_tensor("ln1_g", [E], f32, kind="ExternalInput").ap()
        ln1b_d = nc.dram_tensor("ln1_b", [E], f32, kind="ExternalInput").ap()
    if flags["ln2_gb"]:
        ln2g_d = nc.dram_tensor("ln2_g", [E], f32, kind="ExternalInput").ap()
        ln2b_d = nc.dram_tensor("ln2_b", [E], f32, kind="ExternalInput").ap()
    if flags["b_proj"]:
        bp_d = nc.dram_tensor("b_proj", [E], f32, kind="ExternalInput").ap()
    if flags["b2"]:
        b2_d = nc.dram_tensor("b2", [E], f32, kind="ExternalInput").ap()
    out_d = nc.dram_tensor("out", [T, E], f32, kind="ExternalOutput").ap()

    dbg_outs = {}

    def dbg_tensor(name, shape):
        dbg_outs[name] = nc.dram_tensor(name, shape, f32,
                                        kind="ExternalOutput").ap()
        return dbg_outs[name]

    with tile.TileContext(nc) as tc:
        with (
            tc.tile_pool(name="resid", bufs=TCH) as p_resid,
            tc.tile_pool(name="ht", bufs=ECH) as p_ht,
            tc.tile_pool(name="htok", bufs=3) as p_htok,
            tc.tile_pool(name="small", bufs=6) as p_small,
            tc.tile_pool(name="singles", bufs=1) as p_single,
            tc.tile_pool(name="wsta", bufs=3) as p_wsta,
            tc.tile_pool(name="wmov", bufs=4) as p_wmov,
            tc.tile_pool(name="ps", bufs=8, space="PSUM") as p_ps,
        ):
            # ---- constants ----
            ident = p_single.tile([128, 128], f32, tag="ident", name="ident")
            make_identity(nc, ident[:])
            # tri[k, q] = 1 if k <= q else 0 (upper triangular incl diagonal)
            tri_f = p_single.tile([128, 128], f32, tag="trif", name="trif")
            make_upper_triangular(nc, tri_f[:], val=1.0, diag=True)
            tri = p_single.tile([128, 128], f32r, tag="tri", name="tri")
            nc.vector.tensor_copy(tri[:], tri_f[:])
            ones16 = p_single.tile([128, NH, 1], f32, tag="ones16",
                                   name="ones16")
            nc.vector.memset(ones16[:], 1.0)
            zer384 = p_single.tile([128, 384], f32, tag="zer384",
                                   name="zer384")
            nc.vector.memset(zer384[:], 0.0)
            epst = p_single.tile([128, 1], f32, tag="epst", name="epst")
            nc.vector.memset(epst[:], EPS)
            b1c = p_single.tile([128, FCH], f32, tag="b1c", name="b1c")
            nc.sync.dma_start(b1c[:], b1_d.rearrange("(c p) -> p c", p=128))

            def bcast_row(dram_vec, tag):
                t_ = p_single.tile([128, E], f32, tag=tag, name=tag)
                src = bass.AP(tensor=dram_vec.tensor, offset=dram_vec.offset,
                              ap=[[0, 128]] + list(dram_vec.ap))
                nc.sync.dma_start(t_[:], src)
                return t_

            ln1g_b = bcast_row(ln1g_d, "ln1g") if flags["ln1_gb"] else None
            ln1b_b = bcast_row(ln1b_d, "ln1b") if flags["ln1_gb"] else None
            ln2g_b = bcast_row(ln2g_d, "ln2g") if flags["ln2_gb"] else None
            ln2b_b = bcast_row(ln2b_d, "ln2b") if flags["ln2_gb"] else None
            bp_b = bcast_row(bp_d, "bpb") if flags["b_proj"] else None
            b2_b = bcast_row(b2_d, "b2b") if flags["b2"] else None

            # ---- load x ----
            xt = []
            for tch in range(TCH):
                xt.append(p_resid.tile([128, E], f32, tag="resid",
                                       name="resid"))
                nc.sync.dma_start(xt[tch][:], x_d[128 * tch:128 * (tch + 1), :])

            # ---- layernorm in token layout + PE transpose to [E, T] ----
            def layer_norm_transposed(src_tiles, g_b, b_b):
                ht = [p_ht.tile([128, T], f32r, tag="ht", name="ht")
                      for _ in range(ECH)]
                for tch in range(TCH):
                    xti = src_tiles[tch]
                    st = p_small.tile([128, 2, 6], f32, tag="st", name="st")
                    nc.vector.bn_stats(st[:, 0, :], xti[:, 0:512])
                    nc.vector.bn_stats(st[:, 1, :], xti[:, 512:1024])
                    mv = p_small.tile([128, 2], f32, tag="mv", name="mv")
                    nc.vector.bn_aggr(mv[:], st[:])
                    sq = p_small.tile([128, 1], f32, tag="sq", name="sq")
                    nc.scalar.activation(sq[:], mv[:, 1:2], AF.Sqrt,
                                         bias=epst[:])
                    rsig = p_small.tile([128, 1], f32, tag="rsig", name="rsig")
                    nc.vector.reciprocal(rsig[:], sq[:])
                    h = p_htok.tile([128, E], f32, tag="htok", name="htok")
                    nc.vector.tensor_scalar(h[:], xti[:], mv[:, 0:1],
                                            rsig[:], mybir.AluOpType.subtract,
                                            mybir.AluOpType.mult)
                    if g_b is not None:
                        nc.vector.tensor_mul(h[:], h[:], g_b[:])
                        nc.vector.tensor_add(h[:], h[:], b_b[:])
                    for ec in range(ECH):
                        pst = p_ps.tile([128, 512], f32, tag="ps", name="ps")
                        nc.tensor.transpose(pst[:, 0:128],
                                            h[:, 128 * ec:128 * (ec + 1)],
                                            ident[:])
                        nc.vector.tensor_copy(
                            ht[ec][:, 128 * tch:128 * (tch + 1)],
                            pst[:, 0:128])
                return ht

            ht = layer_norm_transposed(xt, ln1g_b, ln1b_b)

            if sn == 1:
                o = dbg_tensor("dbg_ht", [E, T])
                for ec in range(ECH):
                    nc.sync.dma_start(o[128 * ec:128 * (ec + 1), :],
                                      ht[ec][:].bitcast(f32))

            if sn >= 2:
                # attention-phase pools; closed before the MLP phase
                att_cms = [
                    tc.tile_pool(name="qk", bufs=3),
                    tc.tile_pool(name="vpool", bufs=TCH),
                    tc.tile_pool(name="esc", bufs=6),
                    tc.tile_pool(name="attn", bufs=NPAIR),
                    tc.tile_pool(name="norm", bufs=3),
                ]
                p_qk, p_v, p_esc, p_attn, p_norm = (
                    cm.__enter__() for cm in att_cms)

                # ---- V = h @ wv -> token layout [t, head, 65] + ones col ----
                vt = []
                for tch in range(TCH):
                    v = p_v.tile([128, NH, HD + 1], f32r, tag="v", name="v")
                    nc.vector.tensor_copy(v[:, :, HD:HD + 1], ones16[:])
                    vt.append(v)
                for half in range(2):
                    esl = slice(512 * half, 512 * (half + 1))
                    ys = [p_ps.tile([128, 512], f32, tag="ps", name="ps")
                          for _ in range(TCH)]
                    for ec in range(ECH):
                        wv_t = p_wmov.tile([128, 512], f32r, tag="wmov",
                                           name="wmov")
                        nc.sync.dma_start(wv_t[:],
                                          wv_d[128 * ec:128 * (ec + 1), esl])
                        for tch in range(TCH):
                            nc.tensor.matmul(
                                ys[tch][:],
                                ht[ec][:, 128 * tch:128 * (tch + 1)],
                                wv_t[:], start=(ec == 0),
                                stop=(ec == ECH - 1))
                    for tch in range(TCH):
                        nc.vector.tensor_copy(
                            vt[tch][:, 8 * half:8 * (half + 1), 0:HD],
                            ys[tch][:].rearrange("p (h d) -> p h d", d=HD))

                # ---- per head pair: qT/kT, scores, softmax, att ----
                attn_t = []
                for pair in range(NPAIR if sn >= 3 else 1):
                    cols = slice(128 * pair, 128 * (pair + 1))
                    wq_t = p_wsta.tile([128, ECH, 128], f32r, tag="wsta",
                                       name="wsta")
                    nc.sync.dma_start(
                        wq_t[:],
                        wq_d[:, cols].rearrange("(c p) n -> p c n", p=128))
                    wk_t = p_wsta.tile([128, ECH, 128], f32r, tag="wsta",
                                       name="wsta")
                    nc.sync.dma_start(
                        wk_t[:],
                        wk_d[:, cols].rearrange("(c p) n -> p c n", p=128))
                    qT = p_qk.tile([128, T], f32r, tag="qk", name="qk")
                    kT = p_qk.tile([128, T], f32r, tag="qk", name="qk")
                    for (w_t, dst) in ((wq_t, qT), (wk_t, kT)):
                        for th in range(NTQ):
                            tsl = slice(TQW * th, TQW * (th + 1))
                            ps = p_ps.tile([128, 512], f32, tag="ps",
                                           name="ps")
                            for ec in range(ECH):
                                nc.tensor.matmul(
                                    ps[:], w_t[:, ec, :], ht[ec][:, tsl],
                                    start=(ec == 0), stop=(ec == ECH - 1))
                            nc.vector.tensor_copy(dst[:, tsl], ps[:])

                    if sn == 2 and pair == 0:
                        oq = dbg_tensor("dbg_qT", [128, T])
                        nc.sync.dma_start(oq[:, :], qT[:].bitcast(f32))
                        ok_ = dbg_tensor("dbg_kT", [128, T])
                        nc.sync.dma_start(ok_[:, :], kT[:].bitcast(f32))
                        break

                    att_pair = p_attn.tile([128, T], f32r, tag="attn",
                                           name="attn")
                    attn_t.append(att_pair)
                    for hp in range(2):
                        h_idx = 2 * pair + hp
                        rows = slice(HD * hp, HD * (hp + 1))
                        qh, kh = qT[rows, :], kT[rows, :]
                        for bq in range(NTQ):
                            qsl = slice(TQW * bq, TQW * (bq + 1))
                            nbk = min(TCH, 4 * bq + 4)
                            ps_a = p_ps.tile([128, 512], f32, tag="ps",
                                             name="ps")
                            for bk in range(nbk):
                                ps_s = p_ps.tile([128, 512], f32, tag="ps",
                                                 name="ps")
                                nc.tensor.matmul(
                                    ps_s[:], kh[:, 128 * bk:128 * (bk + 1)],
                                    qh[:, qsl], start=True, stop=True)
                                et = p_esc.tile([128, 512], f32r, tag="esc",
                                                name="esc")
                                d = bk - 4 * bq
                                if d <= 0:
                                    nc.scalar.activation(et[:], ps_s[:],
                                                         AF.Exp, scale=0.125)
                                else:
                                    nc.vector.tensor_copy(
                                        et[:, 0:128 * d],
                                        zer384[:, 0:128 * d])
                                    nc.scalar.activation(
                                        et[:, 128 * d:512],
                                        ps_s[:, 128 * d:512],
                                        AF.Exp, scale=0.125)
                                if d >= 0:
                                    dsl = slice(128 * d, 128 * (d + 1))
                                    nc.vector.tensor_mul(et[:, dsl],
                                                         et[:, dsl], tri[:])
                                nc.tensor.matmul(
                                    ps_a[0:HD + 1, :], vt[bk][:, h_idx, :],
                                    et[:], start=(bk == 0),
                                    stop=(bk == nbk - 1))
                            # normalize by the denominator (row HD of ps_a)
                            rcp = p_norm.tile([HD + 1, 512], f32, tag="rcp",
                                              name="rcp")
                            nc.vector.reciprocal(rcp[HD:HD + 1, :],
                                                 ps_a[HD:HD + 1, :])
                            bct = p_norm.tile([HD, 512], f32, tag="bct",
                                              name="bct")
                            rsl = rcp[HD:HD + 1, :]
                            rap = list(rsl.ap)
                            rbc = bass.AP(tensor=rsl.tensor, offset=rsl.offset,
                                          ap=[rap[0], [0, HD], rap[1]])
                            nc.gpsimd.dma_start(out=bct[:], in_=rbc)
                            if hp == 0:
                                nc.vector.tensor_mul(att_pair[0:HD, qsl],
                                                     ps_a[0:HD, :], bct[:])
                            else:
                                sc = p_norm.tile([HD, 512], f32r,
                                                 tag="oddsc", name="oddsc")
                                nc.vector.tensor_mul(sc[:], ps_a[0:HD, :],
                                                     bct[:])
                                nc.sync.dma_start(att_pair[HD:128, qsl],
                                                  sc[:])

                if sn == 2:
                    o2 = dbg_tensor("dbg_v", [T, NH * (HD + 1)])
                    for tch in range(TCH):
                        nc.sync.dma_start(
                            o2[128 * tch:128 * (tch + 1), :],
                            vt[tch][:].rearrange("p h d -> p (h d)")
                            .bitcast(f32))
                if sn == 3:
                    o = dbg_tensor("dbg_attnT", [E, T])
                    for pr in range(NPAIR):
                        nc.sync.dma_start(o[128 * pr:128 * (pr + 1), :],
                                          attn_t[pr][:].bitcast(f32))

                # ---- out = x + attnT^T @ w_proj (+ b_proj), in-place xt ----
                if sn >= 4:
                    for eo in range(2):
                        esl = slice(512 * eo, 512 * (eo + 1))
                        ys = [p_ps.tile([128, 512], f32, tag="ps", name="ps")
                              for _ in range(TCH)]
                        for pair in range(NPAIR):
                            wp_t = p_wmov.tile([128, 512], f32r, tag="wmov",
                                               name="wmov")
                            nc.sync.dma_start(
                                wp_t[:],
                                wp_d[128 * pair:128 * (pair + 1), esl])
                            for tch in range(TCH):
                                nc.tensor.matmul(
                                    ys[tch][:],
                                    attn_t[pair][:, 128 * tch:128 * (tch + 1)],
                                    wp_t[:], start=(pair == 0),
                                    stop=(pair == NPAIR - 1))
                        for tch in range(TCH):
                            nc.vector.tensor_add(xt[tch][:, esl], ys[tch][:],
                                                 xt[tch][:, esl])
                            if flags["b_proj"]:
                                nc.vector.tensor_add(xt[tch][:, esl],
                                                     xt[tch][:, esl],
                                                     bp_b[:, esl])

                for cm in reversed(att_cms):
                    cm.__exit__(None, None, None)

            if sn == 4:
                o = dbg_tensor("dbg_out1", [T, E])
                for tch in range(TCH):
                    nc.sync.dma_start(o[128 * tch:128 * (tch + 1), :],
                                      xt[tch][:])

            if sn >= 5:
                # ---- LN2 + transpose ----
                h2t = layer_norm_transposed(xt, ln2g_b, ln2b_b)

                # ---- MLP: uT = w1^T h2T; g = gelu(uT+b1); y = g^T w2 ----
                with tc.tile_pool(name="gt", bufs=FCH) as p_gt:
                    for th in range(NTQ):
                        tsl = slice(TQW * th, TQW * (th + 1))
                        gt = []
                        for fc in range(FCH):
                            w1_t = p_wsta.tile([128, ECH, 128], f32r,
                                               tag="wsta", name="wsta")
                            nc.sync.dma_start(
                                w1_t[:], w1_d[:, 128 * fc:128 * (fc + 1)]
                                .rearrange("(c p) n -> p c n", p=128))
                            ps = p_ps.tile([128, 512], f32, tag="ps",
                                           name="ps")
                            for ec in range(ECH):
                                nc.tensor.matmul(
                                    ps[:], w1_t[:, ec, :], h2t[ec][:, tsl],
                                    start=(ec == 0), stop=(ec == ECH - 1))
                            g = p_gt.tile([128, 512], f32r, tag="gt",
                                          name="gt")
                            nc.scalar.activation(g[:], ps[:], AF.Gelu,
                                                 bias=b1c[:, fc:fc + 1])
                            gt.append(g)
                        for eo in range(2):
                            esl = slice(512 * eo, 512 * (eo + 1))
                            ys = [p_ps.tile([128, 512], f32, tag="ps",
                                            name="ps") for _ in range(4)]
                            for fc in range(FCH):
                                w2_t = p_wmov.tile([128, 512], f32r,
                                                   tag="wmov", name="wmov")
                                nc.sync.dma_start(
                                    w2_t[:],
                                    w2_d[128 * fc:128 * (fc + 1), esl])
                                for tl in range(4):
                                    nc.tensor.matmul(
                                        ys[tl][:],
                                        gt[fc][:, 128 * tl:128 * (tl + 1)],
                                        w2_t[:], start=(fc == 0),
                                        stop=(fc == FCH - 1))
                            for tl in range(4):
                                tch = 4 * th + tl
                                nc.vector.tensor_add(xt[tch][:, esl],
                                                     ys[tl][:],
                                                     xt[tch][:, esl])
                                if flags["b2"]:
                                    nc.vector.tensor_add(xt[tch][:, esl],
                                                         xt[tch][:, esl],
                                                         b2_b[:, esl])
                        for tl in range(4):
                            tch = 4 * th + tl
                            nc.sync.dma_start(
                                out_d[128 * tch:128 * (tch + 1), :],
                                xt[tch][:])

    nc.compile()
    return nc, dbg_outs


_CACHE = {}


def _get_program(flags_key, stage="full"):
    key = (flags_key, stage)
    if key not in _CACHE:
        flags = dict(zip(("ln1_gb", "ln2_gb", "b_proj", "b2"), flags_key))
        _CACHE[key] = _build_program(flags, stage)
    return _CACHE[key]


def _flags_for(inputs):
    return {
        "ln1_gb": not (np.all(np.asarray(inputs["ln1_g"]) == 1.0)
                       and np.all(np.asarray(inputs["ln1_b"]) == 0.0)),
        "ln2_gb": not (np.all(np.asarray(inputs["ln2_g"]) == 1.0)
                       and np.all(np.asarray(inputs["ln2_b"]) == 0.0)),
        "b_proj": not np.all(np.asarray(inputs["b_proj"]) == 0.0),
        "b2": not np.all(np.asarray(inputs["b2"]) == 0.0),
    }


def _make_in_maps(inputs, flags, cores):
    common = {}
    for name in ("wq", "wk", "wv", "w_proj", "w1", "w2", "b1"):
        common[name] = np.ascontiguousarray(inputs[name], np.float32)
    for name, flag in (("ln1_g", "ln1_gb"), ("ln1_b", "ln1_gb"),
                       ("ln2_g", "ln2_gb"), ("ln2_b", "ln2_gb"),
                       ("b_proj", "b_proj"), ("b2", "b2")):
        if flags[flag]:
            common[name] = np.ascontiguousarray(inputs[name], np.float32)
    x = np.ascontiguousarray(inputs["x"], np.float32)
    return [{"x": x[c], **common} for c in cores]


def _run(inputs, stage="full", cores=None):
    from concourse.bass_utils import run_bass_kernel_spmd

    if cores is None:
        cores = list(range(NCORES))
    flags = _flags_for(inputs)
    flags_key = tuple(flags[k] for k in ("ln1_gb", "ln2_gb", "b_proj", "b2"))
    nc, dbg = _get_program(flags_key, stage)
    in_maps = _make_in_maps(inputs, flags, cores)
    res = run_bass_kernel_spmd(nc, in_maps, cores)
    return res, dbg


def kernel(**inputs) -> np.ndarray:
    res, _ = _run(inputs)
    return np.stack([res.results[c]["out"] for c in range(NCORES)], axis=0)


# revision 16
# speedup vs baseline: 257.8025x; 257.8025x over previous
"""Trainium2 Bass kernel for a dense transformer block (pre-LN, causal MHA + GELU MLP).

Reference computation (per batch element b, all fp32):
    h   = LN(x; ln1_g, ln1_b)
    q,k,v = h @ wq, h @ wk, h @ wv       (16 heads of dim 64)
    att = softmax(causal(q k^T / 8)) v   -> [T, E]
    out = x + att @ w_proj + b_proj
    mlp = gelu(LN(out; ln2_g, ln2_b) @ w1 + b1) @ w2 + b2
    ret = out + mlp

Sharding: data-parallel over batch. B == 8 == n_cores, one batch element per
NeuronCore, no collectives. Each core runs the identical program on x[b].

Kernel layout strategy (per core):
  - LN1/LN2 computed in token layout [t, E] (free-dim reductions via bn_stats),
    then tiles are PE-transposed to e-partition layout hT/h2T [E, T], which is
    what every matmul needs (contraction dim on partitions).
  - All big matmuls use float32r (TF32-like; 1 cyc/row at N>=256, ~1e-4 relerr).
  - Attention is computed transposed: scoresT[t_k, t_q] = k_h q_h^T so that the
    softmax denominator (sum over keys) can be produced by appending a ones
    column to v_h: attT_psum[65, t_q] = [v_h | 1]^T @ exp(scoresT).  Row 64 is
    the denominator; its reciprocal is partition-broadcast and multiplied in.
  - attnT head-pair tiles (partition = 2x64 head dims) feed the proj matmul as
    the stationary operand directly; proj output lands in token layout and is
    added to x in place (residual).  The MLP's first matmul produces uT [f, t]
    (transposed), so gelu's bias b1 is a per-partition ACT bias, and the second
    matmul consumes gelu(uT) as stationary, producing token-layout output that
    is added to the residual.
"""

import numpy as np

B, T, E = 8, 1024, 1024
NH, HD, FF = 16, 64, 4096
NPAIR = NH // 2          # 8 head pairs (2 heads per 128-partition tile)
EPS = 1e-5
NCORES = 8
TCH = T // 128           # 8 token chunks
ECH = E // 128           # 8 embedding chunks
FCH = FF // 128          # 32 mlp hidden chunks
TQW = 512                # moving-dim width for t
NTQ = T // TQW           # 2

_STAGES = {"ln": 1, "vqk": 2, "attn": 3, "proj": 4, "full": 5}


def _build_program(flags, stage="full"):
    """Build + compile the SPMD Bass program.

    flags: dict of bools controlling optional bias/gain application.
    stage: truncate the program after this phase and emit debug outputs.
    """
    import concourse.bass as bass
    import concourse.tile as tile
    from concourse import bacc, mybir
    from concourse.masks import make_identity, make_upper_triangular

    sn = _STAGES[stage]
    f32 = mybir.dt.float32
    f32r = mybir.dt.float32r
    AF = mybir.ActivationFunctionType

    nc = bacc.Bacc("TRN2", target_bir_lowering=False, debug=False,
                   num_devices=NCORES)

    x_d = nc.dram_tensor("x", [T, E], f32, kind="ExternalInput").ap()
    wq_d = nc.dram_tensor("wq", [E, E], f32r, kind="ExternalInput").ap()
    wk_d = nc.dram_tensor("wk", [E, E], f32r, kind="ExternalInput").ap()
    wv_d = nc.dram_tensor("wv", [E, E], f32r, kind="ExternalInput").ap()
    wp_d = nc.dram_tensor("w_proj", [E, E], f32r, kind="ExternalInput").ap()
    w1_d = nc.dram_tensor("w1", [E, FF], f32r, kind="ExternalInput").ap()
    w2_d = nc.dram_tensor("w2", [FF, E], f32r, kind="ExternalInput").ap()
    b1_d = nc.dram_tensor("b1", [FF], f32, kind="ExternalInput").ap()
    ln1g_d = ln1b_d = ln2g_d = ln2b_d = bp_d = b2_d = None
    if flags["ln1_gb"]:
        ln1g_d = nc.dram_tensor("ln1_g", [E], f32, kind="ExternalInput").ap()
        ln1b_d = nc.dram_tensor("ln1_b", [E], f32, kind="ExternalInput").ap()
    if flags["ln2_gb"]:
        ln2g_d = nc.dram_tensor("ln2_g", [E], f32, kind="ExternalInput").ap()
        ln2b_d = nc.dram_tensor("ln2_b", [E], f32, kind="ExternalInput").ap()
    if flags["b_proj"]:
        bp_d = nc.dram_tensor("b_proj", [E], f32, kind="ExternalInput").ap()
    if flags["b2"]:
        b2_d = nc.dram_tensor("b2", [E], f32, kind="ExternalInput").ap()
    out_d = nc.dram_tensor("out", [T, E], f32, kind="ExternalOutput").ap()

    dbg_outs = {}

    def dbg_tensor(name, shape):
        dbg_outs[name] = nc.dram_tensor(name, shape, f32,
                                        kind="ExternalOutput").ap()
        return dbg_outs[name]

    with tile.TileContext(nc) as tc:
        with (
            tc.tile_pool(name="resid", bufs=TCH) as p_resid,
            tc.tile_pool(name="ht", bufs=ECH) as p_ht,
            tc.tile_pool(name="htok", bufs=3) as p_htok,
            tc.tile_pool(name="small", bufs=6) as p_small,
            tc.tile_pool(name="singles", bufs=1) as p_single,
            tc.tile_pool(name="wsta", bufs=3) as p_wsta,
            tc.tile_pool(name="wmov", bufs=4) as p_wmov,
            tc.tile_pool(name="ps", bufs=8, space="PSUM") as p_ps,
        ):
            # ---- constants ----
            ident = p_single.tile([128, 128], f32, tag="ident", name="ident")
            make_identity(nc, ident[:])
            # tri[k, q] = 1 if k <= q else 0 (upper triangular incl diagonal)
            tri_f = p_single.tile([128, 128], f32, tag="trif", name="trif")
            make_upper_triangular(nc, tri_f[:], val=1.0, diag=True)
            tri = p_single.tile([128, 128], f32r, tag="tri", name="tri")
            nc.vector.tensor_copy(tri[:], tri_f[:])
            ones16 = p_single.tile([128, NH, 1], f32, tag="ones16",
                                   name="ones16")
            nc.vector.memset(ones16[:], 1.0)
            zer384 = p_single.tile([128, 384], f32, tag="zer384",
                                   name="zer384")
            nc.vector.memset(zer384[:], 0.0)
            epst = p_single.tile([128, 1], f32, tag="epst", name="epst")
            nc.vector.memset(epst[:], EPS)
            b1c = p_single.tile([128, FCH], f32, tag="b1c", name="b1c")
            nc.sync.dma_start(b1c[:], b1_d.rearrange("(c p) -> p c", p=128))

            def bcast_row(dram_vec, tag):
                t_ = p_single.tile([128, E], f32, tag=tag, name=tag)
                src = bass.AP(tensor=dram_vec.tensor, offset=dram_vec.offset,
                              ap=[[0, 128]] + list(dram_vec.ap))
                nc.sync.dma_start(t_[:], src)
                return t_

            ln1g_b = bcast_row(ln1g_d, "ln1g") if flags["ln1_gb"] else None
            ln1b_b = bcast_row(ln1b_d, "ln1b") if flags["ln1_gb"] else None
            ln2g_b = bcast_row(ln2g_d, "ln2g") if flags["ln2_gb"] else None
            ln2b_b = bcast_row(ln2b_d, "ln2b") if flags["ln2_gb"] else None
            bp_b = bcast_row(bp_d, "bpb") if flags["b_proj"] else None
            b2_b = bcast_row(b2_d, "b2b") if flags["b2"] else None

            # ---- load x ----
            xt = []
            for tch in range(TCH):
                xt.append(p_resid.tile([128, E], f32, tag="resid",
                                       name="resid"))
                nc.sync.dma_start(xt[tch][:], x_d[128 * tch:128 * (tch + 1), :])

            # ---- layernorm in token layout + PE transpose to [E, T] ----
            def layer_norm_transposed(src_tiles, g_b, b_b):
                ht = [p_ht.tile([128, T], f32r, tag="ht", name="ht")
                      for _ in range(ECH)]
                for tch in range(TCH):
                    xti = src_tiles[tch]
                    st = p_small.tile([128, 2, 6], f32, tag="st", name="st")
                    nc.vector.bn_stats(st[:, 0, :], xti[:, 0:512])
                    nc.vector.bn_stats(st[:, 1, :], xti[:, 512:1024])
                    mv = p_small.tile([128, 2], f32, tag="mv", name="mv")
                    nc.vector.bn_aggr(mv[:], st[:])
                    sq = p_small.tile([128, 1], f32, tag="sq", name="sq")
                    nc.scalar.activation(sq[:], mv[:, 1:2], AF.Sqrt,
                                         bias=epst[:])
                    rsig = p_small.tile([128, 1], f32, tag="rsig", name="rsig")
                    nc.vector.reciprocal(rsig[:], sq[:])
                    h = p_htok.tile([128, E], f32, tag="htok", name="htok")
                    nc.vector.tensor_scalar(h[:], xti[:], mv[:, 0:1],
                                            rsig[:], mybir.AluOpType.subtract,
                                            mybir.AluOpType.mult)
                    if g_b is not None:
                        nc.vector.tensor_mul(h[:], h[:], g_b[:])
                        nc.vector.tensor_add(h[:], h[:], b_b[:])
                    for ec in range(ECH):
                        pst = p_ps.tile([128, 512], f32, tag="ps", name="ps")
                        nc.tensor.transpose(pst[:, 0:128],
                                            h[:, 128 * ec:128 * (ec + 1)],
                                            ident[:])
                        nc.vector.tensor_copy(
                            ht[ec][:, 128 * tch:128 * (tch + 1)],
                            pst[:, 0:128])
                return ht

            ht = layer_norm_transposed(xt, ln1g_b, ln1b_b)

            if sn == 1:
                o = dbg_tensor("dbg_ht", [E, T])
                for ec in range(ECH):
                    nc.sync.dma_start(o[128 * ec:128 * (ec + 1), :],
                                      ht[ec][:].bitcast(f32))

            if sn >= 2:
                # attention-phase pools; closed before the MLP phase
                att_cms = [
                    tc.tile_pool(name="qk", bufs=3),
                    tc.tile_pool(name="vpool", bufs=TCH),
                    tc.tile_pool(name="esc", bufs=6),
                    tc.tile_pool(name="attn", bufs=NPAIR),
                    tc.tile_pool(name="norm", bufs=3),
                ]
                p_qk, p_v, p_esc, p_attn, p_norm = (
                    cm.__enter__() for cm in att_cms)

                # ---- V = h @ wv -> token layout [t, head, 65] + ones col ----
                vt = []
                for tch in range(TCH):
                    v = p_v.tile([128, NH, HD + 1], f32r, tag="v", name="v")
                    nc.vector.tensor_copy(v[:, :, HD:HD + 1], ones16[:])
                    vt.append(v)
                for half in range(2):
                    esl = slice(512 * half, 512 * (half + 1))
                    ys = [p_ps.tile([128, 512], f32, tag="ps", name="ps")
                          for _ in range(TCH)]
                    for ec in range(ECH):
                        wv_t = p_wmov.tile([128, 512], f32r, tag="wmov",
                                           name="wmov")
                        nc.sync.dma_start(wv_t[:],
                                          wv_d[128 * ec:128 * (ec + 1), esl])
                        for tch in range(TCH):
                            nc.tensor.matmul(
                                ys[tch][:],
                                ht[ec][:, 128 * tch:128 * (tch + 1)],
                                wv_t[:], start=(ec == 0),
                                stop=(ec == ECH - 1))
                    for tch in range(TCH):
                        nc.vector.tensor_copy(
                            vt[tch][:, 8 * half:8 * (half + 1), 0:HD],
                            ys[tch][:].rearrange("p (h d) -> p h d", d=HD))

                # ---- per head pair: qT/kT, scores, softmax, att ----
                attn_t = []
                for pair in range(NPAIR if sn >= 3 else 1):
                    cols = slice(128 * pair, 128 * (pair + 1))
                    wq_t = p_wsta.tile([128, ECH, 128], f32r, tag="wsta",
                                       name="wsta")
                    nc.sync.dma_start(
                        wq_t[:],
                        wq_d[:, cols].rearrange("(c p) n -> p c n", p=128))
                    wk_t = p_wsta.tile([128, ECH, 128], f32r, tag="wsta",
                                       name="wsta")
                    nc.sync.dma_start(
                        wk_t[:],
                        wk_d[:, cols].rearrange("(c p) n -> p c n", p=128))
                    qT = p_qk.tile([128, T], f32r, tag="qk", name="qk")
                    kT = p_qk.tile([128, T], f32r, tag="qk", name="qk")
                    for (w_t, dst) in ((wq_t, qT), (wk_t, kT)):
                        for th in range(NTQ):
                            tsl = slice(TQW * th, TQW * (th + 1))
                            ps = p_ps.tile([128, 512], f32, tag="ps",
                                           name="ps")
                            for ec in range(ECH):
                                nc.tensor.matmul(
                                    ps[:], w_t[:, ec, :], ht[ec][:, tsl],
                                    start=(ec == 0), stop=(ec == ECH - 1))
                            nc.vector.tensor_copy(dst[:, tsl], ps[:])

                    if sn == 2 and pair == 0:
                        oq = dbg_tensor("dbg_qT", [128, T])
                        nc.sync.dma_start(oq[:, :], qT[:].bitcast(f32))
                        ok_ = dbg_tensor("dbg_kT", [128, T])
                        nc.sync.dma_start(ok_[:, :], kT[:].bitcast(f32))
                        break

                    att_pair = p_attn.tile([128, T], f32r, tag="attn",
                                           name="attn")
                    attn_t.append(att_pair)
                    for hp in range(2):
                        h_idx = 2 * pair + hp
                        rows = slice(HD * hp, HD * (hp + 1))
                        qh, kh = qT[rows, :], kT[rows, :]
                        for bq in range(NTQ):
                            qsl = slice(TQW * bq, TQW * (bq + 1))
                            nbk = min(TCH, 4 * bq + 4)
                            ps_a = p_ps.tile([128, 512], f32, tag="ps",
                                             name="ps")
                            for bk in range(nbk):
                                ps_s = p_ps.tile([128, 512], f32, tag="ps",
                                                 name="ps")
                                nc.tensor.matmul(
                                    ps_s[:], kh[:, 128 * bk:128 * (bk + 1)],
                                    qh[:, qsl], start=True, stop=True)
                                et = p_esc.tile([128, 512], f32r, tag="esc",
                                                name="esc")
                                d = bk - 4 * bq
                                if d <= 0:
                                    nc.scalar.activation(et[:], ps_s[:],
                                                         AF.Exp, scale=0.125)
                                else:
                                    nc.vector.tensor_copy(
                                        et[:, 0:128 * d],
                                        zer384[:, 0:128 * d])
                                    nc.scalar.activation(
                                        et[:, 128 * d:512],
                                        ps_s[:, 128 * d:512],
                                        AF.Exp, scale=0.125)
                                if d >= 0:
                                    dsl = slice(128 * d, 128 * (d + 1))
                                    nc.vector.tensor_mul(et[:, dsl],
                                                         et[:, dsl], tri[:])
                                nc.tensor.matmul(
                                    ps_a[0:HD + 1, :], vt[bk][:, h_idx, :],
                                    et[:], start=(bk == 0),
                                    stop=(bk == nbk - 1))
                            # normalize by the denominator (row HD of ps_a)
                            rcp = p_norm.tile([HD + 1, 512], f32, tag="rcp",
                                              name="rcp")
                            nc.vector.reciprocal(rcp[HD:HD + 1, :],
                                                 ps_a[HD:HD + 1, :])
                            bct = p_norm.tile([HD, 512], f32, tag="bct",
                                              name="bct")
                            rsl = rcp[HD:HD + 1, :]
                            rap = list(rsl.ap)
                            rbc = bass.AP(tensor=rsl.tensor, offset=rsl.offset,
                                          ap=[rap[0], [0, HD], rap[1]])
                            nc.gpsimd.dma_start(out=bct[:], in_=rbc)
                            if hp == 0:
                                nc.vector.tensor_mul(att_pair[0:HD, qsl],
                                                     ps_a[0:HD, :], bct[:])
                            else:
                                sc = p_norm.tile([HD, 512], f32r,
                                                 tag="oddsc", name="oddsc")
                                nc.vector.tensor_mul(sc[:], ps_a[0:HD, :],
                                                     bct[:])
                                nc.sync.dma_start(att_pair[HD:128, qsl],
                                                  sc[:])

                if sn == 2:
                    o2 = dbg_tensor("dbg_v", [T, NH * (HD + 1)])
                    for tch in range(TCH):
                        nc.sync.dma_start(
                            o2[128 * tch:128 * (tch + 1), :],
                            vt[tch][:].rearrange("p h d -> p (h d)")
                            .bitcast(f32))
                if sn == 3:
                    o = dbg_tensor("dbg_attnT", [E, T])
                    for pr in range(NPAIR):
                        nc.sync.dma_start(o[128 * pr:128 * (pr + 1), :],
                                          attn_t[pr][:].bitcast(f32))

                # ---- out = x + attnT^T @ w_proj (+ b_proj), in-place xt ----
                if sn >= 4:
                    for eo in range(2):
                        esl = slice(512 * eo, 512 * (eo + 1))
                        ys = [p_ps.tile([128, 512], f32, tag="ps", name="ps")
                              for _ in range(TCH)]
                        for pair in range(NPAIR):
                            wp_t = p_wmov.tile([128, 512], f32r, tag="wmov",
                                               name="wmov")
                            nc.sync.dma_start(
                                wp_t[:],
                                wp_d[128 * pair:128 * (pair + 1), esl])
                            for tch in range(TCH):
                                nc.tensor.matmul(
                                    ys[tch][:],
                                    attn_t[pair][:, 128 * tch:128 * (tch + 1)],
                                    wp_t[:], start=(pair == 0),
                                    stop=(pair == NPAIR - 1))
                        for tch in range(TCH):
                            nc.vector.tensor_add(xt[tch][:, esl], ys[tch][:],
                                                 xt[tch][:, esl])
                            if flags["b_proj"]:
                                nc.vector.tensor_add(xt[tch][:, esl],
                                                     xt[tch][:, esl],
                                                     bp_b[:, esl])

                for cm in reversed(att_cms):
                    cm.__exit__(None, None, None)

            if sn == 4:
                o = dbg_tensor("dbg_out1", [T, E])
                for tch in range(TCH):
                    nc.sync.dma_start(o[128 * tch:128 * (tch + 1), :],
                                      xt[tch][:])

            if sn >= 5:
                # ---- LN2 + transpose ----
                h2t = layer_norm_transposed(xt, ln2g_b, ln2b_b)

                # ---- MLP: uT = w1^T h2T; g = gelu(uT+b1); y = g^T w2 ----
                with tc.tile_pool(name="gt", bufs=FCH) as p_gt:
                    for th in range(NTQ):
                        tsl = slice(TQW * th, TQW * (th + 1))
                        gt = []
                        for fc in range(FCH):
                            w1_t = p_wsta.tile([128, ECH, 128], f32r,
                                               tag="wsta", name="wsta")
                            nc.sync.dma_start(
                                w1_t[:], w1_d[:, 128 * fc:128 * (fc + 1)]
                                .rearrange("(c p) n -> p c n", p=128))
                            ps = p_ps.tile([128, 512], f32, tag="ps",
                                           name="ps")
                            for ec in range(ECH):
                                nc.tensor.matmul(
                                    ps[:], w1_t[:, ec, :], h2t[ec][:, tsl],
                                    start=(ec == 0), stop=(ec == ECH - 1))
                            g = p_gt.tile([128, 512], f32r, tag="gt",
                                          name="gt")
                            nc.scalar.activation(g[:], ps[:], AF.Gelu,
                                                 bias=b1c[:, fc:fc + 1])
                            gt.append(g)
                        for eo in range(2):
                            esl = slice(512 * eo, 512 * (eo + 1))
                            ys = [p_ps.tile([128, 512], f32, tag="ps",
                                            name="ps") for _ in range(4)]
                            for fc in range(FCH):
                                w2_t = p_wmov.tile([128, 512], f32r,
                                                   tag="wmov", name="wmov")
                                nc.sync.dma_start(
                                    w2_t[:],
                                    w2_d[128 * fc:128 * (fc + 1), esl])
                                for tl in range(4):
                                    nc.tensor.matmul(
                                        ys[tl][:],
                                        gt[fc][:, 128 * tl:128 * (tl + 1)],
                                        w2_t[:], start=(fc == 0),
                                        stop=(fc == FCH - 1))
                            for tl in range(4):
                                tch = 4 * th + tl
                                nc.vector.tensor_add(xt[tch][:, esl],
                                                     ys[tl][:],
                                                     xt[tch][:, esl])
                                if flags["b2"]:
                                    nc.vector.tensor_add(xt[tch][:, esl],
                                                         xt[tch][:, esl],
                                                         b2_b[:, esl])
                        for tl in range(4):
                            tch = 4 * th + tl
                            nc.sync.dma_start(
                                out_d[128 * tch:128 * (tch + 1), :],
                                xt[tch][:])

    nc.compile()
    return nc, dbg_outs


_CACHE = {}


def _get_program(flags_key, stage="full"):
    key = (flags_key, stage)
    if key not in _CACHE:
        flags = dict(zip(("ln1_gb", "ln2_gb", "b_proj", "b2"), flags_key))
        _CACHE[key] = _build_program(flags, stage)
    return _CACHE[key]


def _flags_for(inputs):
    return {
        "ln1_gb": not (np.all(np.asarray(inputs["ln1_g"]) == 1.0)
                       and np.all(np.asarray(inputs["ln1_b"]) == 0.0)),
        "ln2_gb": not (np.all(np.asarray(inputs["ln2_g"]) == 1.0)
                       and np.all(np.asarray(inputs["ln2_b"]) == 0.0)),
        "b_proj": not np.all(np.asarray(inputs["b_proj"]) == 0.0),
        "b2": not np.all(np.asarray(inputs["b2"]) == 0.0),
    }


def _make_in_maps(inputs, flags, cores):
    common = {}
    for name in ("wq", "wk", "wv", "w_proj", "w1", "w2", "b1"):
        common[name] = np.ascontiguousarray(inputs[name], np.float32)
    for name, flag in (("ln1_g", "ln1_gb"), ("ln1_b", "ln1_gb"),
                       ("ln2_g", "ln2_gb"), ("ln2_b", "ln2_gb"),
                       ("b_proj", "b_proj"), ("b2", "b2")):
        if flags[flag]:
            common[name] = np.ascontiguousarray(inputs[name], np.float32)
    x = np.ascontiguousarray(inputs["x"], np.float32)
    return [{"x": x[c], **common} for c in cores]


def _run(inputs, stage="full", cores=None):
    from concourse.bass_utils import run_bass_kernel_spmd

    if cores is None:
        cores = list(range(NCORES))
    flags = _flags_for(inputs)
    flags_key = tuple(flags[k] for k in ("ln1_gb", "ln2_gb", "b_proj", "b2"))
    nc, dbg = _get_program(flags_key, stage)
    in_maps = _make_in_maps(inputs, flags, cores)
    res = run_bass_kernel_spmd(nc, in_maps, cores)
    return res, dbg


def kernel(**inputs) -> np.ndarray:
    res, _ = _run(inputs)
    return np.stack([res.results[c]["out"] for c in range(NCORES)], axis=0)


def _timed_run(inputs, iters=10, stage="full"):
    """Run the kernel `iters` times back-to-back inside one jit call (chained
    via output-donor buffers) and return (out [B,T,E], seconds_per_iter).

    Used by test.py for device-time measurement; the grading harness only
    calls kernel().
    """
    import time
    import jax
    from jax.experimental.shard_map import shard_map
    from jax.sharding import Mesh, PartitionSpec
    from concourse import bass2jax, mybir
    from concourse.bass2jax import (_bass_exec_p, install_neuronx_cc_hook,
                                    partition_id_tensor)

    install_neuronx_cc_hook()
    flags = _flags_for(inputs)
    flags_key = tuple(flags[k] for k in ("ln1_gb", "ln2_gb", "b_proj", "b2"))
    nc, _ = _get_program(flags_key, stage)
    in_maps = _make_in_maps(inputs, flags, list(range(NCORES)))

    partition_name = (nc.partition_id_tensor.name
                      if nc.partition_id_tensor else None)
    in_names, out_names, out_avals = [], [], []
    for alloc in nc.m.functions[0].allocations:
        if not isinstance(alloc, mybir.MemoryLocationSet):
            continue
        name = alloc.memorylocations[0].name
        if alloc.kind == "ExternalInput":
            if name != partition_name:
                in_names.append(name)
        elif alloc.kind == "ExternalOutput":
            out_names.append(name)
            shape = tuple(alloc.tensor_shape)
            dtype = mybir.dt.np(alloc.dtype)
            out_avals.append(jax.core.ShapedArray(shape, dtype))
    n_params = len(in_names)
    all_names = in_names + out_names
    if partition_name is not None:
        all_names = all_names + [partition_name]

    def _body(*args):
        operands = list(args)
        if partition_name is not None:
            operands.append(partition_id_tensor())
        outs = _bass_exec_p.bind(
            *operands,
            out_avals=tuple(out_avals),
            in_names=tuple(all_names),
            out_names=tuple(out_names),
            lowering_input_output_aliases=(),
            sim_require_finite=True,
            sim_require_nnan=True,
            nc=nc,
        )
        return tuple(outs)

    devices = jax.devices()[:NCORES]
    mesh = Mesh(np.asarray(devices), ("core",))
    n_outs = len(out_names)
    in_specs = (PartitionSpec("core"),) * (n_params + n_outs)
    out_specs = (PartitionSpec("core"),) * n_outs
    fn = jax.jit(shard_map(_body, mesh=mesh, in_specs=in_specs,
                           out_specs=out_specs, check_rep=False),
                 keep_unused=True)

    concat_in = [
        np.concatenate([np.asarray(in_maps[c][nm]) for c in range(NCORES)],
                       axis=0)
        for nm in in_names
    ]
    concat_zeros = [
        np.zeros((NCORES * a.shape[0], *a.shape[1:]), a.dtype)
        for a in out_avals
    ]
    dev_args = [jax.device_put(a) for a in concat_in + concat_zeros]
    out = fn(*dev_args)
    jax.block_until_ready(out)     # warm-up (compile + first run)
    per_call = []
    for _ in range(iters):
        t0 = time.perf_counter()
        out = fn(*dev_args)
        jax.block_until_ready(out)
        per_call.append(time.perf_counter() - t0)
    oi = out_names.index("out")
    res = np.asarray(out[oi]).reshape(NCORES, T, E)
    return res, min(per_call)
